# revision 9
# baseline (speedup 1.0000x reference)
"""Trainium2 Bass kernel for nn_BranchingQNetwork (12-branch dueling Q-MLP).

Strategy: data-parallel over batch (8 cores x 1024 rows). Per core, all 12
branch MLPs run as feature-major GEMM chains (weights stationary, activations
streaming), fp32r matmuls, k-outer single-pass accumulation in 8 PSUM banks
with weights streamed through a small SBUF window. The dueling head
(v + a - mean(a)) is linear, so it is folded into a single [512, 11] weight
matrix on the host.
"""
import sys

sys.path.insert(0, "/opt/trn_rl_repo")

import numpy as np

# problem dims (hardcoded per harness contract)
B = 8192
OBS = 249
NB = 12
NA = 11
NODE = 45
GRP = 17
D0 = 62
D1 = 2048
D2 = 1024
D3 = 512

NCORES = 8
LB = B // NCORES     # local batch per core
BT = 512             # batch tile
NBT = LB // BT
M1 = D1 // 128       # 16 output tiles of layer 1
K2 = D1 // 128       # 16 contraction tiles of layer 2
M2 = D2 // 128       # 8
K3 = D2 // 128       # 8
M3 = D3 // 128       # 4
KH = D3 // 128       # 4
NCH = BT // 128      # 4 batch chunks per batch tile
NAP = 12             # head width padded even (fp32r ISA: N must be even)

_NC_CACHE = {}
LAST_RESULT = None


def _build_nc():
    if "nc" in _NC_CACHE:
        return _NC_CACHE["nc"]
    from concourse import bacc
    import concourse.mybir as mybir
    import concourse.tile as tile

    f32 = mybir.dt.float32
    f32r = mybir.dt.float32r
    Relu = mybir.ActivationFunctionType.Relu
    ADD = mybir.AluOpType.add
    MAX = mybir.AluOpType.max

    nc = bacc.Bacc("TRN2")

    xT_d = nc.declare_dram_parameter("xT", [OBS, LB], f32r, isOutput=False)
    W1_d = nc.declare_dram_parameter("W1p", [NB, D0, D1], f32r, isOutput=False)
    W2_d = nc.declare_dram_parameter("W2p", [NB, K2, 128, D2], f32r, isOutput=False)
    W3_d = nc.declare_dram_parameter("W3p", [NB, K3, 128, D3], f32r, isOutput=False)
    Wq_d = nc.declare_dram_parameter("Wqp", [NB, KH, 128, NAP], f32r, isOutput=False)
    b_d = nc.declare_dram_parameter("bp", [NB, 128, M1 + M2 + M3], f32, isOutput=False)
    bq_d = nc.declare_dram_parameter("bqp", [NB, 128, NAP], f32r, isOutput=False)
    ones_d = nc.declare_dram_parameter("onesp", [1, 128], f32r, isOutput=False)
    out_d = nc.declare_dram_parameter("out", [NB, LB, NA], f32, isOutput=True)

    with tile.TileContext(nc) as tc:
        with (
            tc.tile_pool(name="wp1", bufs=2) as wp1,
            tc.tile_pool(name="wp2", bufs=12) as wp2,
            tc.tile_pool(name="wp3", bufs=8) as wp3,
            tc.tile_pool(name="wpq", bufs=2) as wpq,
            tc.tile_pool(name="bbp", bufs=2) as bbp,
            tc.tile_pool(name="pxp", bufs=3) as pxp,
            tc.tile_pool(name="actp", bufs=1) as actp,
            tc.tile_pool(name="osp", bufs=3) as osp,
            tc.tile_pool(name="psp", bufs=8, space="PSUM") as psp,
        ):
            h1 = actp.tile([128, K2, BT], f32r, tag="h1")
            h2 = actp.tile([128, K3, BT], f32r, tag="h2")
            h3 = actp.tile([128, KH, BT], f32r, tag="h3")

            ones_t = actp.tile([1, 128], f32r, tag="ones")
            nc.sync.dma_start(ones_t[:], ones_d[:])

            iters = [(br, bt) for br in range(NB) for bt in range(NBT)]
            loaded = {}
            pxs = {}

            def load_branch(br):
                eng = nc.sync if br == 0 else nc.scalar
                w1t = wp1.tile([D0, D1], f32r, tag="w1", name=f"w1_{br}")
                eng.dma_start(w1t[:], W1_d[br])
                wqt = wpq.tile([128, KH, NAP], f32r, tag="wq", name=f"wq_{br}")
                eng.dma_start(wqt[:], Wq_d[br].rearrange("k p a -> p k a"))
                btile = bbp.tile([128, M1 + M2 + M3], f32, tag="b", name=f"b_{br}")
                eng.dma_start(btile[:], b_d[br])
                bqt = bbp.tile([1, NAP], f32r, tag="bq", name=f"bq_{br}")
                eng.dma_start(bqt[:], bq_d[br, 0:1, :])
                loaded[br] = (w1t, wqt, btile, bqt)

            def load_px(idx):
                br, bt = iters[idx]
                eng = nc.sync if idx == 0 else nc.scalar
                bsl = slice(bt * BT, (bt + 1) * BT)
                px = pxp.tile([D0, BT], f32r, tag="px", name=f"px_{idx}")
                eng.dma_start(px[0:NODE, :], xT_d[0:NODE, bsl])
                g0 = NODE + GRP * br
                eng.dma_start(px[NODE:D0, :], xT_d[g0:g0 + GRP, bsl])
                pxs[idx] = px

            def drain(dst, ps, bias, j):
                if j % 2 == 0:
                    nc.scalar.activation(dst, ps, Relu, bias=bias, scale=1.0)
                else:
                    nc.vector.tensor_scalar(dst, ps, bias, 0.0, ADD, MAX)

            def drain_split(h, m, ps, bias, j):
                half = BT // 2
                a = h[:, m, 0:half]
                b = h[:, m, half:BT]
                pa = ps[:, 0:half]
                pb = ps[:, half:BT]
                if j % 2 == 0:
                    nc.scalar.activation(a, pa, Relu, bias=bias, scale=1.0)
                    nc.vector.tensor_scalar(b, pb, bias, 0.0, ADD, MAX)
                else:
                    nc.vector.tensor_scalar(a, pa, bias, 0.0, ADD, MAX)
                    nc.scalar.activation(b, pb, Relu, bias=bias, scale=1.0)

            def emit_L1_mm(idx, m):
                br, _ = iters[idx]
                w1t, _, btile, _ = loaded[br]
                ps = psp.tile([128, BT], f32, tag="ps", name=f"l1ps_{idx}_{m}")
                nc.tensor.matmul(
                    ps[:], w1t[:, m * 128:(m + 1) * 128], pxs[idx][:],
                    start=True, stop=True,
                )
                drain(h1[:, m, :], ps[:], btile[:, m:m + 1], m)

            # prologue: first iteration's L1 runs standalone
            load_branch(0)
            load_px(0)
            for m in range(M1):
                emit_L1_mm(0, m)

            for idx, (br, bt) in enumerate(iters):
                w1t, wqt, btile, bqt = loaded[br]
                nxt = idx + 1
                if nxt < len(iters):
                    nbr = iters[nxt][0]
                    if nbr not in loaded:
                        load_branch(nbr)
                    load_px(nxt)

                # ---- L2: [2048 -> 1024], k-outer, 8 psum banks ----
                ps2 = [psp.tile([128, BT], f32, tag="ps", name=f"ps2_{idx}_{_m}")
                       for _m in range(M2)]
                for k in range(K2):
                    w2t = wp2.tile([128, D2], f32r, tag="w2", name=f"w2_{idx}_{k}")
                    nc.sync.dma_start(w2t[:], W2_d[br, k])
                    for m in range(M2):
                        nc.tensor.matmul(
                            ps2[m][:], w2t[:, m * 128:(m + 1) * 128],
                            h1[:, k, :],
                            start=(k == 0), stop=(k == K2 - 1),
                        )
                for m in range(M2):
                    drain_split(h2, m, ps2[m][:], btile[:, M1 + m:M1 + m + 1], m)

                # ---- L3 [1024 -> 512] interleaved with next iteration's L1 ----
                ps3 = [psp.tile([128, BT], f32, tag="ps", name=f"ps3_{idx}_{_m}")
                       for _m in range(M3)]
                for k in range(K3):
                    w3t = wp3.tile([128, D3], f32r, tag="w3", name=f"w3_{idx}_{k}")
                    nc.sync.dma_start(w3t[:], W3_d[br, k])
                    for m in range(M3):
                        nc.tensor.matmul(
                            ps3[m][:], w3t[:, m * 128:(m + 1) * 128],
                            h2[:, k, :],
                            start=(k == 0), stop=(k == K3 - 1),
                        )
                    if nxt < len(iters):
                        emit_L1_mm(nxt, 2 * k)
                        emit_L1_mm(nxt, 2 * k + 1)
                for m in range(M3):
                    drain_split(h3, m, ps3[m][:],
                                btile[:, M1 + M2 + m:M1 + M2 + m + 1], m)

                # ---- head: q = h3 @ Wq + bq, batch-major out ----
                bsl = slice(bt * BT, (bt + 1) * BT)
                ost = osp.tile([128, NCH * NAP], f32, tag="os", name=f"ost_{idx}")
                for c in range(NCH):
                    psh = psp.tile([128, NAP], f32, tag="ps", name=f"psh_{idx}_{c}")
                    for k in range(KH):
                        nc.tensor.matmul(
                            psh[:], h3[:, k, c * 128:(c + 1) * 128],
                            wqt[:, k, :],
                            start=(k == 0), stop=False,
                        )
                    nc.tensor.matmul(
                        psh[:], ones_t[:], bqt[:],
                        start=False, stop=True,
                    )
                    nc.scalar.copy(ost[:, c * NAP:(c + 1) * NAP], psh[:])
                nc.gpsimd.dma_start(
                    out_d[br, bsl, :].rearrange("(c p) a -> p c a", p=128),
                    ost[:].rearrange("p (c a) -> p c a", c=NCH)[:, :, 0:NA],
                )

    nc.compile()
    _NC_CACHE["nc"] = nc
    return nc


def _pack_weights(W1, b1, W2, b2, W3, b3, Wv, bv, Wa, ba):
    f = np.float32
    W1p = np.ascontiguousarray(W1, dtype=f)                      # [12, 62, 2048]
    W2p = np.ascontiguousarray(W2.reshape(NB, K2, 128, D2), f)   # [12,16,128,1024]
    W3p = np.ascontiguousarray(W3.reshape(NB, K3, 128, D3), f)   # [12,8,128,512]
    # fold dueling head: q = h @ (Wv + Wa - mean(Wa)) + (bv + ba - mean(ba))
    Wq = Wv + Wa - Wa.mean(axis=2, keepdims=True)                # [12, 512, 11]
    bq = bv + ba - ba.mean(axis=1, keepdims=True)                # [12, 11]
    Wq = np.concatenate([Wq, np.zeros((NB, D3, NAP - NA), Wq.dtype)], axis=2)
    bq = np.concatenate([bq, np.zeros((NB, NAP - NA), bq.dtype)], axis=1)
    Wqp = np.ascontiguousarray(Wq.reshape(NB, KH, 128, NAP), f)
    bp = np.concatenate(
        [
            b1.reshape(NB, M1, 128).transpose(0, 2, 1),
            b2.reshape(NB, M2, 128).transpose(0, 2, 1),
            b3.reshape(NB, M3, 128).transpose(0, 2, 1),
        ],
        axis=2,
    ).astype(f)                                                  # [12, 128, 28]
    bqp = np.ascontiguousarray(
        np.broadcast_to(bq[:, None, :], (NB, 128, NAP)), f
    )
    return W1p, W2p, W3p, Wqp, bp, bqp


def kernel(x, W1, b1, W2, b2, W3, b3, Wv, bv, Wa, ba):
    global LAST_RESULT
    from concourse.bass_utils import run_bass_kernel_spmd

    x = np.asarray(x, np.float32)
    args = [np.asarray(a, np.float32) for a in (W1, b1, W2, b2, W3, b3, Wv, bv, Wa, ba)]
    W1p, W2p, W3p, Wqp, bp, bqp = _pack_weights(*args)

    nc = _build_nc()
    ones_row = np.ones((1, 128), np.float32)
    in_maps = []
    for c in range(NCORES):
        xT = np.ascontiguousarray(x[c * LB:(c + 1) * LB].T)      # [249, 1024]
        in_maps.append({
            "xT": xT,
            "W1p": W1p, "W2p": W2p, "W3p": W3p, "Wqp": Wqp,
            "bp": bp, "bqp": bqp, "onesp": ones_row,
        })

    res = run_bass_kernel_spmd(nc, in_maps, list(range(NCORES)))
    LAST_RESULT = res

    out = np.empty((NB, B, NA), np.float32)
    for c in range(NCORES):
        out[:, c * LB:(c + 1) * LB, :] = res.results[c]["out"]
    return out
